# revision 1
# baseline (speedup 1.0000x reference)
"""Trainium2 Bass kernel for nn_MultiHeadDuelingDQN (8-core SPMD).

Model (B=256, STATE=26240, H=512, R=4000, N=64 heads, M=10):
    h  = relu(relu(x@W1+b1)@W2+b2)
    q_cache = h@Wvc+bvc + (h@Wac+bac) - mean_R(h@Wac+bac)
    q_assoc = per-head dueling over M (local means)
    q_rec   = S - mean_R(S),  S = sum_n (h@Wru[n]+bru[n])   [exact rewrite:
              rec_global has zero row-mean, so the reference's second mean
              subtraction is a no-op and S never needs the [B,N,R] tensor]

Sharding (8 cores):
  - fc1: contraction (STATE) split 8 ways; partial h1_pre [256,512] exchanged
    via AllToAll (cheapest collective here) + local 8-way sum of the core's
    32 batch rows (b1/8 folded pre-exchange, relu post-sum); fc2 computed on
    the 32 local rows, then AllGather replicates h2 to all cores.
  - rec/cache: R split 8 ways (500 cols/core); the sum over heads becomes a
    DVE reduction over repacked contiguous Wru supertiles ([128, 4*500] per
    DMA, heads interleaved innermost), then one small matmul h @ W_sum.
    Row-means over the full R use a tiny [128,4] AllGather + local reduce.
  - assoc heads: split 8 ways (8 heads/core), fully local; augmented matmul
    [Wau | Wvu | Wvc] -> [adv_assoc | val_n | value_c] in one pass.
Engine split: PE does transposes + all matmuls; DVE is dedicated to the Wru
stream reduction; ACT (scalar) does PSUM->SBUF copies, relus, row-sums
(accum_out) and mean subtraction (Identity+bias), plus non-stream DMA
dispatch; Sync dispatches the Wru stream; GpSimd runs collectives and small
SBUF elementwise ops.

kernel(**inputs) takes full unsharded inputs, returns full [256, 8640].
"""
import os
os.environ.setdefault("NEURON_RT_DBG_RDH_CC", "0")

import numpy as np

import concourse.bass as bass
import concourse.mybir as mybir
import concourse.tile as tile
from concourse import bacc
from concourse import bass_utils
from concourse.bass import ts
from concourse.masks import make_identity

NC = 8
B, H, STATE, R, NH, M = 256, 512, 26240, 4000, 64, 10
KPC_RAW = STATE // NC          # 3280
KCH = 26                       # k-chunks of 128 per core (padded)
KPC = KCH * 128                # 3328
RPC = R // NC                  # 500
HPC = NH // NC                 # 8 heads per core
AUG = HPC * (M + 1) + 1        # 89 = [8x(10 adv + 1 val)] + value_c
GRP = 4                        # heads per wru supertile
NGRP = NH // GRP               # 8 supertiles per k-chunk
W1GROUPS = [(0, 4), (4, 4), (8, 4), (12, 4), (16, 4), (20, 4), (24, 2)]
F32 = mybir.dt.float32
RELU = mybir.ActivationFunctionType.Relu
COPY = mybir.ActivationFunctionType.Copy
IDENT = mybir.ActivationFunctionType.Identity
ADD = mybir.AluOpType.add
SUB = mybir.AluOpType.subtract


def build_program(wru_bufs=7):
    nc = bacc.Bacc("TRN2", target_bir_lowering=False, debug=False, num_devices=NC)

    # ---- per-core I/O ----
    xs = nc.dram_tensor("xs", [B, KPC], F32, kind="ExternalInput").ap()
    w1g = [nc.dram_tensor(f"w1g{i}", [128, 512 * L], F32, kind="ExternalInput").ap()
           for i, (_, L) in enumerate(W1GROUPS)]
    b1 = nc.dram_tensor("b1", [H], F32, kind="ExternalInput").ap()
    w2 = nc.dram_tensor("w2", [H, H], F32, kind="ExternalInput").ap()
    b2 = nc.dram_tensor("b2", [H], F32, kind="ExternalInput").ap()
    wac = nc.dram_tensor("wac", [H, RPC], F32, kind="ExternalInput").ap()
    bac = nc.dram_tensor("bac", [RPC], F32, kind="ExternalInput").ap()
    # repacked r-major: [kc, grp, p, r*8+gi] = Wru[grp*8+gi, kc*128+p, r0+r]
    wru = nc.dram_tensor("wru", [4, NGRP, 128, GRP * RPC], F32,
                         kind="ExternalInput").ap()
    bru = nc.dram_tensor("bru", [NH, RPC], F32, kind="ExternalInput").ap()
    wau = nc.dram_tensor("wau", [HPC, H, M], F32, kind="ExternalInput").ap()
    bau = nc.dram_tensor("bau", [HPC, M], F32, kind="ExternalInput").ap()
    wvu = nc.dram_tensor("wvu", [HPC, H], F32, kind="ExternalInput").ap()
    bvu = nc.dram_tensor("bvu", [HPC], F32, kind="ExternalInput").ap()
    wvc = nc.dram_tensor("wvc", [H], F32, kind="ExternalInput").ap()
    bvc = nc.dram_tensor("bvc", [1], F32, kind="ExternalInput").ap()

    out_cache = nc.dram_tensor("out_cache", [B, RPC], F32, kind="ExternalOutput").ap()
    out_rec = nc.dram_tensor("out_rec", [B, RPC], F32, kind="ExternalOutput").ap()
    out_assoc = nc.dram_tensor("out_assoc", [B, HPC * M], F32, kind="ExternalOutput").ap()

    with tile.TileContext(nc) as tc:
        with (
            tc.tile_pool(name="cst", bufs=1) as cst,
            tc.tile_pool(name="sb", bufs=1) as sb,
            tc.tile_pool(name="w1p", bufs=3) as w1p,
            tc.tile_pool(name="wrup", bufs=wru_bufs) as wrup,
            tc.tile_pool(name="partp", bufs=1) as partp,
            tc.tile_pool(name="ps", bufs=2, space="PSUM") as ps,
            tc.tile_pool(name="psfc", bufs=2, space="PSUM") as psfc,
            tc.tile_pool(name="dram", bufs=1, space="DRAM") as dram,
        ):
            ident = cst.tile([128, 128], F32, tag="ident")
            make_identity(nc, ident)
            ones8 = cst.tile([1, 128], F32, tag="ones8")
            nc.vector.memset(ones8, 1.0 / NC)
            ones1 = cst.tile([1, 128], F32, tag="ones1")
            nc.vector.memset(ones1, 1.0)
            ones64 = cst.tile([64, 128], F32, tag="ones64")
            nc.vector.memset(ones64, 1.0)

            # x slice loads first (they gate the whole trunk)
            x_sb = []
            for bt in range(2):
                xsb = sb.tile([128, KPC], F32, tag=f"x_sb{bt}", name=f"x_sb{bt}")
                nc.scalar.dma_start(xsb, xs[ts(bt, 128), :])
                x_sb.append(xsb)

            # ---------- Phase D emit first: Wru stream + head pre-sum (DVE) ----
            # (emission order does not pin execution order, but DMAs here only
            # depend on pool slots so they can start immediately)
            acc = [sb.tile([128, RPC], F32, tag=f"acc{k}", name=f"acc{k}")
                   for k in range(4)]
            for kc in range(4):
                for g in range(NGRP):
                    wt = wrup.tile([128, GRP * RPC], F32, tag="wru", name=f"wru_t{kc}_{g}")
                    nc.sync.dma_start(wt, wru[kc, g])
                    view = bass.AP(wt.tensor, wt.offset,
                                   [wt.ap[0], [GRP, RPC], [1, GRP]])
                    if g == 0:
                        nc.vector.tensor_reduce(acc[kc], view,
                                                axis=mybir.AxisListType.X, op=ADD)
                    else:
                        part = partp.tile([128, RPC], F32, tag="part",
                                          name=f"part{kc}_{g}")
                        nc.vector.tensor_reduce(part, view,
                                                axis=mybir.AxisListType.X, op=ADD)
                        nc.vector.tensor_add(acc[kc], acc[kc], part)
            bru_sb = sb.tile([64, RPC], F32, tag="bru_sb")
            nc.scalar.dma_start(bru_sb, bru)

            # ---------- Phase A: trunk ----------
            # fc1 pipeline: per k-chunk transpose x (both halves) then matmul
            b1row = cst.tile([1, H], F32, tag="b1row")
            nc.scalar.dma_start(b1row, b1.rearrange("(a h) -> a h", a=1))
            h1_ps = [psfc.tile([128, H], F32, tag="fc", name=f"h1_ps{bt}")
                     for bt in range(2)]
            for bt in range(2):  # fold b1/8 first, opens the accumulation group
                nc.tensor.matmul(h1_ps[bt], ones8, b1row, start=True, stop=False)
            for gi, (base, L) in enumerate(W1GROUPS):
                w1t = w1p.tile([128, 512 * L], F32, tag="w1g", name=f"w1t{gi}")
                nc.scalar.dma_start(w1t, w1g[gi])
                for j in range(L):
                    kc = base + j
                    xTt = sb.tile([128, B], F32, tag="xTrot", bufs=10,
                                  name=f"xT{kc}")
                    for bt in range(2):
                        pt = ps.tile([128, 128], F32, tag="small", bufs=4,
                                     name=f"ptx{bt}_{kc}")
                        nc.tensor.transpose(pt, x_sb[bt][:, ts(kc, 128)], ident)
                        nc.scalar.copy(xTt[:, ts(bt, 128)], pt)
                    for bt in range(2):
                        nc.tensor.matmul(h1_ps[bt], xTt[:, ts(bt, 128)],
                                         w1t[:, ts(j, 512)],
                                         start=False, stop=(kc == KCH - 1))

            # AllToAll h1_pre [256,512]: rank c receives 8 partials of its
            # 32 batch rows, then sums them locally (cheaper than RS here)
            BPC = B // NC  # 32 batch rows per core
            rs_in = dram.tile([B, H], F32, tag="rs_in")
            rs_out = dram.tile([B, H], F32, tag="rs_out")
            for bt in range(2):
                t = sb.tile([128, H], F32, tag=f"h1c{bt}", name=f"h1c{bt}")
                nc.scalar.copy(t, h1_ps[bt])
                nc.scalar.dma_start(rs_in[ts(bt, 128), :], t)
            nc.gpsimd.collective_compute(
                "AllToAll", mybir.AluOpType.bypass,
                replica_groups=[list(range(NC))],
                ins=[rs_in.opt()], outs=[rs_out.opt()],
            )
            # readback the 8 partials and tree-sum on GpSimd
            parts = []
            for i in range(NC):
                pti = sb.tile([BPC, H], F32, tag=f"h1p{i}", name=f"h1p{i}")
                nc.scalar.dma_start(pti, rs_out[ts(i, BPC), :])
                parts.append(pti)
            h1rs = sb.tile([BPC, H], F32, tag="h1rs")
            nc.gpsimd.tensor_add(h1rs, parts[0], parts[1])
            for i in range(2, NC):
                nc.gpsimd.tensor_add(h1rs, h1rs, parts[i])
            h1s = sb.tile([BPC, H], F32, tag="h1s")
            nc.scalar.activation(h1s, h1rs, RELU)
            h1cT = []
            for kc in range(4):
                pt = ps.tile([128, BPC], F32, tag="small", bufs=4, name=f"pth{kc}")
                nc.tensor.transpose(pt, h1s[:, ts(kc, 128)], ident[0:BPC, 0:BPC])
                t = sb.tile([128, BPC], F32, tag=f"h1cT{kc}", name=f"h1cT{kc}")
                nc.scalar.copy(t, pt)
                h1cT.append(t)
            w2t = []
            for kc in range(4):
                t = sb.tile([128, H], F32, tag=f"w2_{kc}", name=f"w2_{kc}")
                nc.scalar.dma_start(t, w2[ts(kc, 128), :])
                w2t.append(t)
            b2row = cst.tile([1, H], F32, tag="b2row")
            nc.scalar.dma_start(b2row, b2.rearrange("(a h) -> a h", a=1))
            h2_ps = psfc.tile([BPC, H], F32, tag="fc", name="h2_ps")
            nc.tensor.matmul(h2_ps, ones1[:, 0:BPC], b2row, start=True, stop=False)
            for kc in range(4):
                nc.tensor.matmul(h2_ps, h1cT[kc], w2t[kc],
                                 start=False, stop=(kc == 3))
            h2s = sb.tile([BPC, H], F32, tag="h2s")
            nc.scalar.activation(h2s, h2_ps, RELU)
            ag_in = dram.tile([BPC, H], F32, tag="ag_in")
            ag_out = dram.tile([B, H], F32, tag="ag_out")
            nc.scalar.dma_start(ag_in, h2s)
            nc.gpsimd.collective_compute(
                "AllGather", mybir.AluOpType.bypass,
                replica_groups=[list(range(NC))],
                ins=[ag_in.opt()], outs=[ag_out.opt()],
            )
            # h2 [256, 512] -> hT chunks [128(h2), 256(b)]
            hT = [sb.tile([128, B], F32, tag=f"hT{kc}", name=f"hT{kc}")
                  for kc in range(4)]
            for bt in range(2):
                h2g = sb.tile([128, H], F32, tag=f"h2g{bt}", name=f"h2g{bt}")
                nc.scalar.dma_start(h2g, ag_out[ts(bt, 128), :])
                for kc in range(4):
                    pt = ps.tile([128, 128], F32, tag="small", bufs=4, name=f"ptg{bt}_{kc}")
                    nc.tensor.transpose(pt, h2g[:, ts(kc, 128)], ident)
                    nc.scalar.copy(hT[kc][:, ts(bt, 128)], pt)

            # ---------- Phase B: assoc heads (augmented [adv|val|value_c]) ------
            aug_w = []
            for kc in range(4):
                t = cst.tile([128, AUG], F32, tag=f"aug_w{kc}", name=f"aug_w{kc}")
                grid = t[:, 0:HPC * (M + 1)].rearrange("p (n u) -> p n u", u=M + 1)
                nc.scalar.dma_start(
                    grid[:, :, 0:M],
                    wau[:, ts(kc, 128), :].rearrange("n k m -> k n m"))
                nc.scalar.dma_start(
                    grid[:, :, M:M + 1],
                    wvu[:, ts(kc, 128)].rearrange("n (k u) -> k n u", u=1))
                nc.scalar.dma_start(
                    t[:, AUG - 1:AUG],
                    wvc[ts(kc, 128)].rearrange("(k u) -> k u", u=1))
                aug_w.append(t)
            aug_b = cst.tile([1, AUG], F32, tag="aug_b")
            bgrid = aug_b[:, 0:HPC * (M + 1)].rearrange("p (n u) -> p n u", u=M + 1)
            nc.scalar.dma_start(bgrid[:, :, 0:M], bau.rearrange("n (a m) -> a n m", a=1))
            nc.scalar.dma_start(bgrid[:, :, M:M + 1],
                              bvu.rearrange("(a n u) -> a n u", a=1, u=1))
            nc.scalar.dma_start(aug_b[:, AUG - 1:AUG], bvc.rearrange("(a u) -> a u", a=1))

            value_sb = []
            junkA = sb.tile([128, M], F32, tag="junkA")
            for bt in range(2):
                psA = ps.tile([128, AUG], F32, tag="wide", name=f"psA{bt}")
                nc.tensor.matmul(psA, ones1, aug_b, start=True, stop=False)
                for kc in range(4):
                    nc.tensor.matmul(psA, hT[kc][:, ts(bt, 128)], aug_w[kc],
                                     start=False, stop=(kc == 3))
                # copy to SBUF so GpSimd can finalize (no DVE involvement)
                psA_sb = sb.tile([128, AUG], F32, tag=f"psAsb{bt}", name=f"psAsb{bt}")
                nc.scalar.copy(psA_sb, psA)
                advs = psA_sb[:, 0:HPC * (M + 1)].rearrange("p (n u) -> p n u", u=M + 1)
                # per-head -mean over M via ACT accum_out (free-axis sum)
                negm = sb.tile([128, HPC], F32, tag=f"negmA{bt}", name=f"negmA{bt}")
                for n in range(HPC):
                    nc.scalar.activation(junkA, advs[:, n, 0:M], COPY,
                                         scale=-1.0 / M,
                                         accum_out=negm[:, n:n + 1])
                tmp = sb.tile([128, HPC], F32, tag=f"tmpA{bt}", name=f"tmpA{bt}")
                nc.gpsimd.tensor_add(tmp, advs[:, :, M], negm)
                q = sb.tile([128, HPC * M], F32, tag=f"qA{bt}", name=f"qA{bt}")
                nc.gpsimd.tensor_tensor(
                    out=q.rearrange("p (n m) -> p n m", m=M),
                    in0=advs[:, :, 0:M],
                    in1=tmp.broadcast_to([128, HPC, M]),
                    op=ADD)
                nc.scalar.dma_start(out_assoc[ts(bt, 128), :], q)
                value_sb.append(psA_sb[:, AUG - 1:AUG])

            # ---------- Phase C: cache head (R-slice) ----------
            ar2_in = sb.tile([128, 4], F32, tag="ar2_in")
            wac_t = []
            for kc in range(4):
                t = sb.tile([128, RPC], F32, tag=f"wac{kc}", name=f"wac{kc}")
                nc.scalar.dma_start(t, wac[ts(kc, 128), :])
                wac_t.append(t)
            bac_sb = cst.tile([1, RPC], F32, tag="bac_sb")
            nc.scalar.dma_start(bac_sb, bac.rearrange("(a r) -> a r", a=1))
            adv_c_sb = []
            for bt in range(2):
                psC = ps.tile([128, RPC], F32, tag="wide", name=f"psC{bt}")
                nc.tensor.matmul(psC, ones1, bac_sb, start=True, stop=False)
                for kc in range(4):
                    nc.tensor.matmul(psC, hT[kc][:, ts(bt, 128)], wac_t[kc],
                                     start=False, stop=(kc == 3))
                t = sb.tile([128, RPC], F32, tag=f"advc{bt}", name=f"advc{bt}")
                # copy + row-sum in one ACT pass (accum_out)
                nc.scalar.activation(t, psC, COPY,
                                     accum_out=ar2_in[:, bt:bt + 1])
                adv_c_sb.append(t)

            # ---------- S = hT.T @ W_sum (+ sum_n bru fold), row-sums ----------
            psS, s_sb = [], []
            for bt in range(2):
                t = ps.tile([128, RPC], F32, tag="wide", name=f"psS{bt}")
                nc.tensor.matmul(t, ones64, bru_sb, start=True, stop=False)
                for kc in range(4):
                    nc.tensor.matmul(t, hT[kc][:, ts(bt, 128)], acc[kc],
                                     start=False, stop=(kc == 3))
                st = sb.tile([128, RPC], F32, tag=f"ssb{bt}", name=f"ssb{bt}")
                nc.scalar.activation(st, t, COPY,
                                     accum_out=ar2_in[:, 2 + bt:3 + bt])
                s_sb.append(st)
                psS.append(t)

            # ---------- Phase E: tiny AllReduce of row-sums, finalize ----------
            ar2_din = dram.tile([128, 4], F32, tag="ar2_din")
            ar2_dout = dram.tile([NC * 128, 4], F32, tag="ar2_dout")
            nc.scalar.dma_start(ar2_din, ar2_in)
            nc.gpsimd.collective_compute(
                "AllGather", mybir.AluOpType.bypass,
                replica_groups=[list(range(NC))],
                ins=[ar2_din.opt()], outs=[ar2_dout.opt()],
            )
            # one strided readback [128, (g,c)] then a single X-reduce over g
            rall = sb.tile([128, NC * 4], F32, tag="rall")
            nc.scalar.dma_start(rall, ar2_dout.rearrange("(g p) c -> p g c", p=128))
            rview = bass.AP(rall.tensor, rall.offset,
                            [rall.ap[0], [1, 4], [4, NC]])
            ar2_sb = sb.tile([128, 4], F32, tag="ar2_sb")
            nc.vector.tensor_reduce(ar2_sb, rview, axis=mybir.AxisListType.X, op=ADD)
            negmeans = sb.tile([128, 4], F32, tag="negmeans")
            nc.scalar.activation(negmeans, ar2_sb, COPY, scale=-1.0 / R)

            for bt in range(2):
                vm = sb.tile([128, 1], F32, tag=f"vm{bt}", name=f"vm{bt}")
                nc.gpsimd.tensor_add(vm, value_sb[bt], negmeans[:, bt:bt + 1])
                qc = sb.tile([128, RPC], F32, tag=f"qc{bt}", name=f"qc{bt}")
                nc.scalar.activation(qc, adv_c_sb[bt], IDENT, bias=vm, scale=1.0)
                nc.scalar.dma_start(out_cache[ts(bt, 128), :], qc)

                qr = sb.tile([128, RPC], F32, tag=f"qr{bt}", name=f"qr{bt}")
                nc.scalar.activation(qr, s_sb[bt], IDENT,
                                     bias=negmeans[:, 2 + bt:3 + bt], scale=1.0)
                nc.scalar.dma_start(out_rec[ts(bt, 128), :], qr)

    nc.compile()
    return nc


_CACHED = None


def _get_program():
    global _CACHED
    if _CACHED is None:
        _CACHED = build_program()
    return _CACHED


def make_in_maps(x, W1, b1, W2, b2, Wvc, bvc, Wac, bac, Wvu, bvu, Wau, bau, Wru, bru):
    f = np.float32
    x = np.asarray(x, f)
    W1 = np.asarray(W1, f)
    Wru = np.asarray(Wru, f)
    in_maps = []
    for c in range(NC):
        k0 = c * KPC_RAW
        xs = np.zeros((B, KPC), f)
        xs[:, :KPC_RAW] = x[:, k0:k0 + KPC_RAW]
        w1s = np.zeros((KPC, H), f)
        w1s[:KPC_RAW] = W1[k0:k0 + KPC_RAW]
        w1r = w1s.reshape(KCH, 128, H)
        r0 = c * RPC
        h0 = c * HPC
        # wru repack r-major: [kc, grp, p, r*8+gi] = Wru[grp*8+gi, kc*128+p, r0+r]
        ws = Wru[:, :, r0:r0 + RPC]                       # [64, 512, 500]
        a = ws.reshape(NGRP, GRP, 4, 128, RPC)            # [grp, gi, kc, p, r]
        wru_p = np.ascontiguousarray(a.transpose(2, 0, 3, 4, 1)).reshape(
            4, NGRP, 128, GRP * RPC)
        m = {
            "xs": xs,
            "b1": np.asarray(b1, f), "w2": np.asarray(W2, f), "b2": np.asarray(b2, f),
            "wac": np.ascontiguousarray(np.asarray(Wac, f)[:, r0:r0 + RPC]),
            "bac": np.ascontiguousarray(np.asarray(bac, f)[r0:r0 + RPC]),
            "wru": wru_p,
            "bru": np.ascontiguousarray(np.asarray(bru, f)[:, r0:r0 + RPC]),
            "wau": np.ascontiguousarray(np.asarray(Wau, f)[h0:h0 + HPC]),
            "bau": np.ascontiguousarray(np.asarray(bau, f)[h0:h0 + HPC]),
            "wvu": np.ascontiguousarray(np.asarray(Wvu, f)[h0:h0 + HPC]),
            "bvu": np.ascontiguousarray(np.asarray(bvu, f)[h0:h0 + HPC]),
            "wvc": np.ascontiguousarray(np.asarray(Wvc, f).reshape(H)),
            "bvc": np.asarray(bvc, f).reshape(1),
        }
        for gi, (base, L) in enumerate(W1GROUPS):
            m[f"w1g{gi}"] = np.ascontiguousarray(
                w1r[base:base + L].transpose(1, 0, 2)).reshape(128, L * 512)
        in_maps.append(m)
    return in_maps


def assemble(results):
    q = np.empty((B, 2 * R + NH * M), np.float32)
    for c in range(NC):
        r0 = c * RPC
        a0 = c * HPC * M
        q[:, r0:r0 + RPC] = results[c]["out_cache"]
        q[:, R + r0:R + r0 + RPC] = results[c]["out_rec"]
        q[:, 2 * R + a0:2 * R + a0 + HPC * M] = results[c]["out_assoc"]
    return q


def run(in_maps, **kw):
    nc = _get_program()
    return bass_utils.run_bass_kernel_spmd(nc, in_maps, core_ids=list(range(NC)), **kw)


def kernel(**inputs):
    in_maps = make_in_maps(**{k: np.asarray(v) for k, v in inputs.items()})
    res = run(in_maps)
    return assemble(res.results)



# revision 2
# speedup vs baseline: 1.6442x; 1.6442x over previous
"""Trainium2 Bass kernel for nn_MultiHeadDuelingDQN (8-core SPMD), v2.

Model (B=256, STATE=26240, H=512, R=4000, N=64 heads, M=10):
    h  = relu(relu(x@W1+b1)@W2+b2)
    q_cache = h@Wvc+bvc + (h@Wac+bac) - mean_R(h@Wac+bac)
    q_assoc = per-head dueling over M (local means)
    q_rec   = S - mean_R(S),  S = sum_n (h@Wru[n]+bru[n])   [exact rewrite:
              rec_global has zero row-mean so the second mean is a no-op]

v2 design (vs v1 baseline at ~344us):
  - Everything streams in bf16 (halves HBM traffic; Wru 65.5->32.8 MB/core)
    and all matmuls run bf16 (1 cyc/row vs fp32's 4).
  - All weights are host-repacked into their exact SBUF image [128, X], so
    every load is one contiguous-per-partition DMA and the device does ZERO
    transposes: trunk computes h1T/h2T [h, b] directly (lhsT = natural W1/W2
    k-major chunks, rhs = host-transposed xT), and the per-user heads consume
    h2T as lhsT directly.
  - b1/b2 become per-partition ACT biases (free relu+bias+cast in one op).
  - fc1 is contraction-sharded (STATE/8 per core); partials are combined with
    ONE AllReduce [512,256] fp32 (vs v1's AllToAll + AllGather), fully hidden
    under the Wru stream. fc2 runs redundantly on every core (1.7us).
  - Wru head-sum: DVE-only stream reduction over GRP=16-head-interleaved
    supertiles [128, 500*16] bf16; bf16 in/out hits the DVE 2x_1P mode.
    DVE does nothing else during the stream; all small finalize work runs on
    ACT/GpSimd so it can't queue behind the stream reduces.
  - Full-R row-means need a cross-core sum; split into TWO tiny AllGathers:
    cache-head one early (hidden under stream), rec-head one at stream end
    (the only exposed collective, ~7us tail).
  - psS accumulation interleaved by k-chunk so only the last acc[3] matmul
    (0.2us) waits for the stream end.

kernel(**inputs) takes full unsharded fp32 inputs, returns full [256, 8640].
"""
import os
os.environ.setdefault("NEURON_RT_DBG_RDH_CC", "0")

import numpy as np
import ml_dtypes

import concourse.bass as bass
import concourse.mybir as mybir
import concourse.tile as tile
from concourse import bacc
from concourse import bass_utils
from concourse.bass import ts

NC = 8
B, H, STATE, R, NH, M = 256, 512, 26240, 4000, 64, 10
KPC_RAW = STATE // NC          # 3280
KCH = 26                       # k-chunks of 128 per core (padded)
KPC = KCH * 128                # 3328
RPC = R // NC                  # 500
HPC = NH // NC                 # 8 heads per core
AUG = HPC * (M + 1) + 1        # 89 = [8x(10 adv + 1 val)] + value_c
GRP = 16                       # heads per wru supertile
NGRP = NH // GRP               # 4 supertiles per k-chunk
F32 = mybir.dt.float32
BF16 = mybir.dt.bfloat16
RELU = mybir.ActivationFunctionType.Relu
COPY = mybir.ActivationFunctionType.Copy
IDENT = mybir.ActivationFunctionType.Identity
ADD = mybir.AluOpType.add
BF = ml_dtypes.bfloat16


def build_program(wru_bufs=4):
    nc = bacc.Bacc("TRN2", target_bir_lowering=False, debug=False, num_devices=NC)

    # ---- per-core I/O (all host-packed to exact SBUF images) ----
    xt = nc.dram_tensor("xt", [128, KCH * B], BF16, kind="ExternalInput").ap()
    w1 = nc.dram_tensor("w1", [128, KCH * H], BF16, kind="ExternalInput").ap()
    b1p = nc.dram_tensor("b1p", [128, 4], F32, kind="ExternalInput").ap()
    w2p = nc.dram_tensor("w2p", [128, 4 * H], BF16, kind="ExternalInput").ap()
    b2p = nc.dram_tensor("b2p", [128, 4], F32, kind="ExternalInput").ap()
    wacp = nc.dram_tensor("wacp", [128, 4 * RPC], BF16, kind="ExternalInput").ap()
    bacp = nc.dram_tensor("bacp", [1, RPC], BF16, kind="ExternalInput").ap()
    # supertiles: [kc, g, p, r*GRP+gi] = Wru[g*GRP+gi, kc*128+p, r0+r]
    wrup = nc.dram_tensor("wrup", [4, NGRP, 128, GRP * RPC], BF16,
                          kind="ExternalInput").ap()
    brup = nc.dram_tensor("brup", [NH, RPC], BF16, kind="ExternalInput").ap()
    augp = nc.dram_tensor("augp", [128, 4 * AUG], BF16, kind="ExternalInput").ap()
    augb = nc.dram_tensor("augb", [1, AUG], BF16, kind="ExternalInput").ap()

    out_cache = nc.dram_tensor("out_cache", [B, RPC], F32, kind="ExternalOutput").ap()
    out_rec = nc.dram_tensor("out_rec", [B, RPC], F32, kind="ExternalOutput").ap()
    out_assoc = nc.dram_tensor("out_assoc", [B, HPC * M], F32, kind="ExternalOutput").ap()

    with tile.TileContext(nc) as tc, \
         nc.allow_low_precision(reason="bf16 stream reduction; gate is 2e-2"):
        with (
            tc.tile_pool(name="cst", bufs=1) as cst,
            tc.tile_pool(name="sb", bufs=1) as sb,
            tc.tile_pool(name="wrupool", bufs=wru_bufs) as wrupool,
            tc.tile_pool(name="partp", bufs=2) as partp,
            tc.tile_pool(name="psfc", bufs=4, space="PSUM") as psfc,
            tc.tile_pool(name="psw", bufs=2, space="PSUM") as psw,
            tc.tile_pool(name="pss", bufs=2, space="PSUM") as pss,
            tc.tile_pool(name="dram", bufs=1, space="DRAM") as dram,
        ):
            ones1 = cst.tile([1, 128], BF16, tag="ones1")
            nc.vector.memset(ones1, 1.0)
            ones64 = cst.tile([64, 128], BF16, tag="ones64")
            nc.vector.memset(ones64, 1.0)

            # ---------- sync queue: trunk inputs then the Wru stream ----------
            w1sb = cst.tile([128, KCH * H], BF16, tag="w1sb")
            nc.sync.dma_start(w1sb, w1)
            xsb = cst.tile([128, KCH * B], BF16, tag="xsb")
            nc.sync.dma_start(xsb, xt)

            # Wru stream + head pre-sum on DVE (DVE does nothing else until
            # the stream is done)
            acc = [sb.tile([128, RPC], BF16, tag=f"acc{k}", name=f"acc{k}")
                   for k in range(4)]
            for kc in range(4):
                for g in range(NGRP):
                    wt = wrupool.tile([128, GRP * RPC], BF16, tag="wru",
                                      name=f"wru_t{kc}_{g}")
                    nc.sync.dma_start(wt, wrup[kc, g])
                    view = bass.AP(wt.tensor, wt.offset,
                                   [wt.ap[0], [GRP, RPC], [1, GRP]])
                    if g == 0:
                        nc.vector.tensor_reduce(acc[kc], view,
                                                axis=mybir.AxisListType.X, op=ADD)
                    else:
                        part = partp.tile([128, RPC], BF16, tag="part",
                                          name=f"part{kc}_{g}")
                        nc.vector.tensor_reduce(part, view,
                                                axis=mybir.AxisListType.X, op=ADD)
                        nc.vector.tensor_add(acc[kc], acc[kc], part)

            # ---------- scalar queue: small loads ----------
            b1sb = cst.tile([128, 4], F32, tag="b1sb")
            nc.scalar.dma_start(b1sb, b1p)
            b2sb = cst.tile([128, 4], F32, tag="b2sb")
            nc.scalar.dma_start(b2sb, b2p)
            w2sb = cst.tile([128, 4 * H], BF16, tag="w2sb")
            nc.scalar.dma_start(w2sb, w2p)
            wacsb = cst.tile([128, 4 * RPC], BF16, tag="wacsb")
            nc.scalar.dma_start(wacsb, wacp)
            bacsb = cst.tile([1, RPC], BF16, tag="bacsb")
            nc.scalar.dma_start(bacsb, bacp)
            augsb = cst.tile([128, 4 * AUG], BF16, tag="augsb")
            nc.scalar.dma_start(augsb, augp)
            augbsb = cst.tile([1, AUG], BF16, tag="augbsb")
            nc.scalar.dma_start(augbsb, augb)
            brusb = cst.tile([64, RPC], BF16, tag="brusb")
            nc.scalar.dma_start(brusb, brup)

            # ---------- fc1: h1T[ht] = sum_k W1[k, ht]·xT[k, b] ----------
            ps1 = []
            for ht in range(4):
                p = psfc.tile([128, B], F32, tag="fc", name=f"ps1_{ht}")
                for kc in range(KCH):
                    nc.tensor.matmul(p, w1sb[:, kc * H + ht * 128:kc * H + (ht + 1) * 128],
                                     xsb[:, ts(kc, B)],
                                     start=(kc == 0), stop=(kc == KCH - 1))
                ps1.append(p)
            h1loc = sb.tile([128, 4, B], F32, tag="h1loc")
            for ht in range(4):
                nc.scalar.copy(h1loc[:, ht, :], ps1[ht])

            # ---------- ONE AllReduce of fc1 partials (hidden under stream) --
            ar_in = dram.tile([128, 4, B], F32, tag="ar_in")
            ar_out = dram.tile([128, 4, B], F32, tag="ar_out", addr_space="Shared")
            nc.scalar.dma_start(ar_in, h1loc)
            nc.gpsimd.collective_compute(
                "AllReduce", ADD,
                replica_groups=[list(range(NC))],
                ins=[ar_in.opt()], outs=[ar_out.opt()],
            )
            h1r = sb.tile([128, 4, B], F32, tag="h1r")
            nc.scalar.dma_start(h1r, ar_out)
            h1T = []
            for ht in range(4):
                t = sb.tile([128, B], BF16, tag=f"h1T{ht}", name=f"h1T{ht}")
                nc.scalar.activation(t, h1r[:, ht, :], RELU,
                                     bias=b1sb[:, ht:ht + 1])
                h1T.append(t)

            # ---------- fc2 (redundant on every core, trivial) ----------
            hT = []
            for ht in range(4):
                p = psfc.tile([128, B], F32, tag="fc", name=f"ps2_{ht}")
                for kc in range(4):
                    nc.tensor.matmul(p, w2sb[:, kc * H + ht * 128:kc * H + (ht + 1) * 128],
                                     h1T[kc],
                                     start=(kc == 0), stop=(kc == 3))
                t = sb.tile([128, B], BF16, tag=f"hT{ht}", name=f"hT{ht}")
                nc.scalar.activation(t, p, RELU, bias=b2sb[:, ht:ht + 1])
                hT.append(t)

            # ---------- assoc heads + value_c (augmented matmul) ----------
            ar1_in = sb.tile([128, 2], F32, tag="ar1_in")
            ar2_in = sb.tile([128, 2], F32, tag="ar2_in")
            value_sb = []
            junkA = sb.tile([128, M], F32, tag="junkA")
            for bt in range(2):
                psA = psw.tile([128, AUG], F32, tag="wide", name=f"psA{bt}")
                nc.tensor.matmul(psA, ones1, augbsb, start=True, stop=False)
                for kc in range(4):
                    nc.tensor.matmul(psA, hT[kc][:, ts(bt, 128)],
                                     augsb[:, kc * AUG:(kc + 1) * AUG],
                                     start=False, stop=(kc == 3))
                psA_sb = sb.tile([128, AUG], F32, tag=f"psAsb{bt}", name=f"psAsb{bt}")
                nc.scalar.copy(psA_sb, psA)
                advs = psA_sb[:, 0:HPC * (M + 1)].rearrange("p (n u) -> p n u", u=M + 1)
                negm = sb.tile([128, HPC], F32, tag=f"negmA{bt}", name=f"negmA{bt}")
                for n in range(HPC):
                    nc.scalar.activation(junkA, advs[:, n, 0:M], COPY,
                                         scale=-1.0 / M,
                                         accum_out=negm[:, n:n + 1])
                tmp = sb.tile([128, HPC], F32, tag=f"tmpA{bt}", name=f"tmpA{bt}")
                nc.gpsimd.tensor_add(tmp, advs[:, :, M], negm)
                q = sb.tile([128, HPC * M], F32, tag=f"qA{bt}", name=f"qA{bt}")
                nc.gpsimd.tensor_tensor(
                    out=q.rearrange("p (n m) -> p n m", m=M),
                    in0=advs[:, :, 0:M],
                    in1=tmp.broadcast_to([128, HPC, M]),
                    op=ADD)
                nc.scalar.dma_start(out_assoc[ts(bt, 128), :], q)
                value_sb.append(psA_sb[:, AUG - 1:AUG])

            # ---------- cache head (R-slice) + early row-sums ----------
            adv_c_sb = []
            for bt in range(2):
                psC = psw.tile([128, RPC], F32, tag="wide", name=f"psC{bt}")
                nc.tensor.matmul(psC, ones1, bacsb, start=True, stop=False)
                for kc in range(4):
                    nc.tensor.matmul(psC, hT[kc][:, ts(bt, 128)],
                                     wacsb[:, ts(kc, RPC)],
                                     start=False, stop=(kc == 3))
                t = sb.tile([128, RPC], F32, tag=f"advc{bt}", name=f"advc{bt}")
                nc.scalar.activation(t, psC, COPY,
                                     accum_out=ar1_in[:, bt:bt + 1])
                adv_c_sb.append(t)

            # tiny AllGather #1 (cache path; hidden under the stream)
            ag1_din = dram.tile([128, 2], F32, tag="ag1_din")
            ag1_dout = dram.tile([NC * 128, 2], F32, tag="ag1_dout",
                                 addr_space="Shared")
            nc.scalar.dma_start(ag1_din, ar1_in)
            nc.gpsimd.collective_compute(
                "AllGather", mybir.AluOpType.bypass,
                replica_groups=[list(range(NC))],
                ins=[ag1_din.opt()], outs=[ag1_dout.opt()],
            )
            rall1 = sb.tile([128, NC * 2], F32, tag="rall1")
            nc.scalar.dma_start(rall1, ag1_dout.rearrange("(g p) c -> p g c", p=128))
            junk8 = sb.tile([128, NC], F32, tag="junk8")
            negm1 = sb.tile([128, 2], F32, tag="negm1")
            for bt in range(2):
                gv = bass.AP(rall1.tensor, rall1.offset + bt,
                             [rall1.ap[0], [2, NC]])
                nc.scalar.activation(junk8, gv, COPY, scale=-1.0 / R,
                                     accum_out=negm1[:, bt:bt + 1])
            for bt in range(2):
                vm = sb.tile([128, 1], F32, tag=f"vm{bt}", name=f"vm{bt}")
                nc.gpsimd.tensor_add(vm, value_sb[bt], negm1[:, bt:bt + 1])
                qc = sb.tile([128, RPC], F32, tag=f"qc{bt}", name=f"qc{bt}")
                nc.scalar.activation(qc, adv_c_sb[bt], IDENT, bias=vm, scale=1.0)
                nc.scalar.dma_start(out_cache[ts(bt, 128), :], qc)

            # ---------- S = hT.T @ acc (+ sum_n bru), interleaved by kc ------
            psS = []
            for bt in range(2):
                p = pss.tile([128, RPC], F32, tag="s", name=f"psS{bt}")
                nc.tensor.matmul(p, ones64, brusb, start=True, stop=False)
                psS.append(p)
            for kc in range(4):
                for bt in range(2):
                    nc.tensor.matmul(psS[bt], hT[kc][:, ts(bt, 128)], acc[kc],
                                     start=False, stop=(kc == 3))
            s_sb = []
            for bt in range(2):
                st = sb.tile([128, RPC], F32, tag=f"ssb{bt}", name=f"ssb{bt}")
                nc.scalar.activation(st, psS[bt], COPY,
                                     accum_out=ar2_in[:, bt:bt + 1])
                s_sb.append(st)

            # tiny AllGather #2 (rec path; the only exposed tail collective)
            ag2_din = dram.tile([128, 2], F32, tag="ag2_din")
            ag2_dout = dram.tile([NC * 128, 2], F32, tag="ag2_dout",
                                 addr_space="Shared")
            nc.scalar.dma_start(ag2_din, ar2_in)
            nc.gpsimd.collective_compute(
                "AllGather", mybir.AluOpType.bypass,
                replica_groups=[list(range(NC))],
                ins=[ag2_din.opt()], outs=[ag2_dout.opt()],
            )
            rall2 = sb.tile([128, NC * 2], F32, tag="rall2")
            nc.scalar.dma_start(rall2, ag2_dout.rearrange("(g p) c -> p g c", p=128))
            negm2 = sb.tile([128, 2], F32, tag="negm2")
            for bt in range(2):
                gv = bass.AP(rall2.tensor, rall2.offset + bt,
                             [rall2.ap[0], [2, NC]])
                nc.scalar.activation(junk8, gv, COPY, scale=-1.0 / R,
                                     accum_out=negm2[:, bt:bt + 1])
            for bt in range(2):
                qr = sb.tile([128, RPC], F32, tag=f"qr{bt}", name=f"qr{bt}")
                nc.scalar.activation(qr, s_sb[bt], IDENT,
                                     bias=negm2[:, bt:bt + 1], scale=1.0)
                nc.scalar.dma_start(out_rec[ts(bt, 128), :], qr)

    nc.compile()
    return nc


_CACHED = None


def _get_program():
    global _CACHED
    if _CACHED is None:
        _CACHED = build_program()
    return _CACHED


def make_in_maps(x, W1, b1, W2, b2, Wvc, bvc, Wac, bac, Wvu, bvu, Wau, bau, Wru, bru):
    f = np.float32
    x_bf = np.asarray(x, f).astype(BF)                    # [B, STATE]
    W1_bf = np.asarray(W1, f).astype(BF)                  # [STATE, H]
    W2_bf = np.asarray(W2, f).astype(BF)
    Wac_bf = np.asarray(Wac, f).astype(BF)
    Wru_bf = np.asarray(Wru, f).astype(BF)                # [64, 512, 4000]
    Wau_f = np.asarray(Wau, f)
    Wvu_f = np.asarray(Wvu, f)
    Wvc_f = np.asarray(Wvc, f).reshape(H)
    bau_f = np.asarray(bau, f)
    bvu_f = np.asarray(bvu, f)
    bvc_f = np.asarray(bvc, f).reshape(1)
    bru_bf = np.asarray(bru, f).astype(BF)
    bac_f = np.asarray(bac, f)
    b1_f = np.asarray(b1, f)
    b2_f = np.asarray(b2, f)

    # w2: [h1, h2] -> [p, kc*H + h2]
    w2p = np.ascontiguousarray(
        W2_bf.reshape(4, 128, H).transpose(1, 0, 2)).reshape(128, 4 * H)
    b1p = np.ascontiguousarray(b1_f.reshape(4, 128).T)
    b2p = np.ascontiguousarray(b2_f.reshape(4, 128).T)

    in_maps = []
    for c in range(NC):
        k0 = c * KPC_RAW
        r0 = c * RPC
        h0 = c * HPC
        # xT: [p, kc*B + b] = x[b, k0 + kc*128 + p]
        xs = np.zeros((KPC, B), BF)
        xs[:KPC_RAW] = x_bf[:, k0:k0 + KPC_RAW].T
        xt = np.ascontiguousarray(
            xs.reshape(KCH, 128, B).transpose(1, 0, 2)).reshape(128, KCH * B)
        # w1: [p, kc*H + h] = W1[k0 + kc*128 + p, h]
        w1s = np.zeros((KPC, H), BF)
        w1s[:KPC_RAW] = W1_bf[k0:k0 + KPC_RAW]
        w1p_ = np.ascontiguousarray(
            w1s.reshape(KCH, 128, H).transpose(1, 0, 2)).reshape(128, KCH * H)
        # wac: [p, kc*RPC + r] = Wac[kc*128 + p, r0 + r]
        wacp_ = np.ascontiguousarray(
            Wac_bf[:, r0:r0 + RPC].reshape(4, 128, RPC).transpose(1, 0, 2)
        ).reshape(128, 4 * RPC)
        # wru supertiles: [kc, g, p, r*GRP + gi] = Wru[g*GRP+gi, kc*128+p, r0+r]
        ws = Wru_bf[:, :, r0:r0 + RPC]                    # [64, 512, 500]
        a = ws.reshape(NGRP, GRP, 4, 128, RPC)            # [g, gi, kc, p, r]
        wru_p = np.ascontiguousarray(a.transpose(2, 0, 3, 4, 1)).reshape(
            4, NGRP, 128, GRP * RPC)
        # aug weights: [k, n*(M+1)+m | n*(M+1)+M | 88]
        aug_full = np.empty((H, AUG), f)
        aug_full[:, 0:HPC * (M + 1)] = np.concatenate(
            [Wau_f[h0:h0 + HPC].transpose(1, 0, 2),            # [H, 8, 10]
             Wvu_f[h0:h0 + HPC].T[:, :, None]], axis=2         # [H, 8, 1]
        ).reshape(H, HPC * (M + 1))
        aug_full[:, AUG - 1] = Wvc_f
        augp_ = np.ascontiguousarray(
            aug_full.astype(BF).reshape(4, 128, AUG).transpose(1, 0, 2)
        ).reshape(128, 4 * AUG)
        augb_ = np.empty((1, AUG), f)
        augb_[0, 0:HPC * (M + 1)] = np.concatenate(
            [bau_f[h0:h0 + HPC], bvu_f[h0:h0 + HPC, None]], axis=1
        ).reshape(HPC * (M + 1))
        augb_[0, AUG - 1] = bvc_f[0]
        m = {
            "xt": xt,
            "w1": w1p_,
            "b1p": b1p,
            "w2p": w2p,
            "b2p": b2p,
            "wacp": wacp_,
            "bacp": np.ascontiguousarray(bac_f[None, r0:r0 + RPC]).astype(BF),
            "wrup": wru_p,
            "brup": np.ascontiguousarray(bru_bf[:, r0:r0 + RPC]),
            "augp": augp_,
            "augb": augb_.astype(BF),
        }
        in_maps.append(m)
    return in_maps


def assemble(results):
    q = np.empty((B, 2 * R + NH * M), np.float32)
    for c in range(NC):
        r0 = c * RPC
        a0 = c * HPC * M
        q[:, r0:r0 + RPC] = results[c]["out_cache"]
        q[:, R + r0:R + r0 + RPC] = results[c]["out_rec"]
        q[:, 2 * R + a0:2 * R + a0 + HPC * M] = results[c]["out_assoc"]
    return q


def run(in_maps, **kw):
    nc = _get_program()
    return bass_utils.run_bass_kernel_spmd(nc, in_maps, core_ids=list(range(NC)), **kw)


def kernel(**inputs):
    in_maps = make_in_maps(**{k: np.asarray(v) for k, v in inputs.items()})
    res = run(in_maps)
    return assemble(res.results)


# revision 5
# speedup vs baseline: 1.7737x; 1.0788x over previous
"""Trainium2 Bass kernel for nn_MultiHeadDuelingDQN (8-core SPMD), v2.

Model (B=256, STATE=26240, H=512, R=4000, N=64 heads, M=10):
    h  = relu(relu(x@W1+b1)@W2+b2)
    q_cache = h@Wvc+bvc + (h@Wac+bac) - mean_R(h@Wac+bac)
    q_assoc = per-head dueling over M (local means)
    q_rec   = S - mean_R(S),  S = sum_n (h@Wru[n]+bru[n])   [exact rewrite:
              rec_global has zero row-mean so the second mean is a no-op]

v2 design (vs v1 baseline at ~344us):
  - Everything streams in bf16 (halves HBM traffic; Wru 65.5->32.8 MB/core)
    and all matmuls run bf16 (1 cyc/row vs fp32's 4).
  - All weights are host-repacked into their exact SBUF image [128, X], so
    every load is one contiguous-per-partition DMA and the device does ZERO
    transposes: trunk computes h1T/h2T [h, b] directly (lhsT = natural W1/W2
    k-major chunks, rhs = host-transposed xT), and the per-user heads consume
    h2T as lhsT directly.
  - b1/b2 become per-partition ACT biases (free relu+bias+cast in one op).
  - fc1 is contraction-sharded (STATE/8 per core); partials are combined with
    ONE AllReduce [512,256] fp32 (vs v1's AllToAll + AllGather), fully hidden
    under the Wru stream. fc2 runs redundantly on every core (1.7us).
  - Wru head-sum: DVE-only stream reduction over GRP=16-head-interleaved
    supertiles [128, 500*16] bf16; bf16 in/out hits the DVE 2x_1P mode.
    DVE does nothing else during the stream; all small finalize work runs on
    ACT/GpSimd so it can't queue behind the stream reduces.
  - Full-R row-means need a cross-core sum; split into TWO tiny AllGathers:
    cache-head one early (hidden under stream), rec-head one at stream end
    (the only exposed collective, ~7us tail).
  - psS accumulation interleaved by k-chunk so only the last acc[3] matmul
    (0.2us) waits for the stream end.

kernel(**inputs) takes full unsharded fp32 inputs, returns full [256, 8640].
"""
import os
os.environ.setdefault("NEURON_RT_DBG_RDH_CC", "0")

import numpy as np
import ml_dtypes

import concourse.bass as bass
import concourse.mybir as mybir
import concourse.tile as tile
from concourse import bacc
from concourse import bass_utils
from concourse.bass import ts

NC = 8
B, H, STATE, R, NH, M = 256, 512, 26240, 4000, 64, 10
KPC_RAW = STATE // NC          # 3280
KCH = 26                       # k-chunks of 128 per core (padded)
KPC = KCH * 128                # 3328
RPC = R // NC                  # 500
HPC = NH // NC                 # 8 heads per core
AUG = HPC * (M + 1) + 1        # 89 = [8x(10 adv + 1 val)] + value_c
GRP = 16                       # heads per wru supertile
NGRP = NH // GRP               # 4 supertiles per k-chunk
F32 = mybir.dt.float32
BF16 = mybir.dt.bfloat16
RELU = mybir.ActivationFunctionType.Relu
COPY = mybir.ActivationFunctionType.Copy
IDENT = mybir.ActivationFunctionType.Identity
ADD = mybir.AluOpType.add
BF = ml_dtypes.bfloat16


def build_program(wru_bufs=4):
    nc = bacc.Bacc("TRN2", target_bir_lowering=False, debug=False, num_devices=NC)

    # ---- per-core I/O (all host-packed to exact SBUF images) ----
    xt = nc.dram_tensor("xt", [128, KCH * B], BF16, kind="ExternalInput").ap()
    w1 = nc.dram_tensor("w1", [128, KCH * H], BF16, kind="ExternalInput").ap()
    b1p = nc.dram_tensor("b1p", [128, 4], F32, kind="ExternalInput").ap()
    w2p = nc.dram_tensor("w2p", [128, 4 * H], BF16, kind="ExternalInput").ap()
    b2p = nc.dram_tensor("b2p", [128, 4], F32, kind="ExternalInput").ap()
    wacp = nc.dram_tensor("wacp", [128, 4 * RPC], BF16, kind="ExternalInput").ap()
    bacp = nc.dram_tensor("bacp", [1, RPC], BF16, kind="ExternalInput").ap()
    # supertiles: [kc, g, p, r*GRP+gi] = Wru[g*GRP+gi, kc*128+p, r0+r]
    wrup = nc.dram_tensor("wrup", [4, NGRP, 128, GRP * RPC], BF16,
                          kind="ExternalInput").ap()
    brup = nc.dram_tensor("brup", [NH, RPC], BF16, kind="ExternalInput").ap()
    augp = nc.dram_tensor("augp", [128, 4 * AUG], BF16, kind="ExternalInput").ap()
    augb = nc.dram_tensor("augb", [1, AUG], BF16, kind="ExternalInput").ap()

    out_cache = nc.dram_tensor("out_cache", [B, RPC], F32, kind="ExternalOutput").ap()
    out_rec = nc.dram_tensor("out_rec", [B, RPC], F32, kind="ExternalOutput").ap()
    out_assoc = nc.dram_tensor("out_assoc", [B, HPC * M], F32, kind="ExternalOutput").ap()

    with tile.TileContext(nc) as tc, \
         nc.allow_low_precision(reason="bf16 stream reduction; gate is 2e-2"):
        with (
            tc.tile_pool(name="cst", bufs=1) as cst,
            tc.tile_pool(name="sb", bufs=1) as sb,
            tc.tile_pool(name="wrupool", bufs=wru_bufs) as wrupool,
            tc.tile_pool(name="partp", bufs=2) as partp,
            tc.tile_pool(name="psfc", bufs=4, space="PSUM") as psfc,
            tc.tile_pool(name="psw", bufs=2, space="PSUM") as psw,
            tc.tile_pool(name="pss", bufs=2, space="PSUM") as pss,
            tc.tile_pool(name="dram", bufs=1, space="DRAM") as dram,
        ):
            ones1 = cst.tile([1, 128], BF16, tag="ones1")
            nc.vector.memset(ones1, 1.0)
            ones64 = cst.tile([64, 128], BF16, tag="ones64")
            nc.vector.memset(ones64, 1.0)

            # ---------- sync queue: trunk inputs then the Wru stream ----------
            # x first, then w1 in 4 chunk-groups so fc1 can start early and
            # keep the PE continuously busy (HAM warm)
            xsb = cst.tile([128, KCH * B], BF16, tag="xsb")
            nc.sync.dma_start(xsb, xt)
            W1G = [(0, 7), (7, 7), (14, 6), (20, 6)]
            w1g = []
            for gi, (base, L) in enumerate(W1G):
                t = cst.tile([128, L * H], BF16, tag=f"w1g{gi}", name=f"w1g{gi}")
                nc.sync.dma_start(t, w1[:, base * H:(base + L) * H])
                w1g.append(t)

            # Wru stream + head pre-sum: pairwise TT-add tree on DVE (bf16
            # unit-stride TT hits the 2x_1P mode; tensor_reduce does not).
            # Supertile = 16 head-blocks of 500; tree halves 4 times.
            acc = [sb.tile([128, RPC], BF16, tag=f"acc{k}", name=f"acc{k}")
                   for k in range(4)]
            for kc in range(4):
                for g in range(NGRP):
                    wt = wrupool.tile([128, GRP * RPC], BF16, tag="wru",
                                      name=f"wru_t{kc}_{g}")
                    nc.sync.dma_start(wt, wrup[kc, g])
                    t1 = partp.tile([128, 8 * RPC], BF16, tag="t1",
                                    name=f"t1_{kc}_{g}")
                    nc.vector.tensor_add(t1, wt[:, 0:8 * RPC], wt[:, 8 * RPC:16 * RPC])
                    t2 = partp.tile([128, 4 * RPC], BF16, tag="t2",
                                    name=f"t2_{kc}_{g}")
                    nc.vector.tensor_add(t2, t1[:, 0:4 * RPC], t1[:, 4 * RPC:8 * RPC])
                    t3 = partp.tile([128, 2 * RPC], BF16, tag="t3",
                                    name=f"t3_{kc}_{g}")
                    nc.vector.tensor_add(t3, t2[:, 0:2 * RPC], t2[:, 2 * RPC:4 * RPC])
                    if g == 0:
                        nc.vector.tensor_add(acc[kc], t3[:, 0:RPC], t3[:, RPC:2 * RPC])
                    else:
                        part = partp.tile([128, RPC], BF16, tag="part",
                                          name=f"part{kc}_{g}")
                        nc.vector.tensor_add(part, t3[:, 0:RPC], t3[:, RPC:2 * RPC])
                        nc.vector.tensor_add(acc[kc], acc[kc], part)

            # ---------- scalar queue: small loads ----------
            b1sb = cst.tile([128, 4], F32, tag="b1sb")
            nc.scalar.dma_start(b1sb, b1p)
            b2sb = cst.tile([128, 4], F32, tag="b2sb")
            nc.scalar.dma_start(b2sb, b2p)
            w2sb = cst.tile([128, 4 * H], BF16, tag="w2sb")
            nc.scalar.dma_start(w2sb, w2p)
            wacsb = cst.tile([128, 4 * RPC], BF16, tag="wacsb")
            nc.scalar.dma_start(wacsb, wacp)
            bacsb = cst.tile([1, RPC], BF16, tag="bacsb")
            nc.scalar.dma_start(bacsb, bacp)
            augsb = cst.tile([128, 4 * AUG], BF16, tag="augsb")
            nc.scalar.dma_start(augsb, augp)
            augbsb = cst.tile([1, AUG], BF16, tag="augbsb")
            nc.scalar.dma_start(augbsb, augb)
            brusb = cst.tile([64, RPC], BF16, tag="brusb")
            nc.scalar.dma_start(brusb, brup)

            # ---------- fc1: h1T[ht] = sum_k W1[k, ht]·xT[k, b] ----------
            # kc-outer so matmuls start as soon as x + the first w1 group land
            ps1 = [psfc.tile([128, B], F32, tag="fc", name=f"ps1_{ht}")
                   for ht in range(4)]
            for gi, (base, L) in enumerate(W1G):
                for j in range(L):
                    kc = base + j
                    for ht in range(4):
                        nc.tensor.matmul(
                            ps1[ht],
                            w1g[gi][:, j * H + ht * 128:j * H + (ht + 1) * 128],
                            xsb[:, ts(kc, B)],
                            start=(kc == 0), stop=(kc == KCH - 1))
            h1loc = sb.tile([128, 4, B], BF16, tag="h1loc")
            for ht in range(4):
                nc.scalar.copy(h1loc[:, ht, :], ps1[ht])

            # ---------- ONE AllReduce of fc1 partials (hidden under stream) --
            # bf16 wire halves the collective time; numerics fine (pre-relu)
            ar_in = dram.tile([128, 4, B], BF16, tag="ar_in")
            ar_out = dram.tile([128, 4, B], BF16, tag="ar_out", addr_space="Shared")
            nc.scalar.dma_start(ar_in, h1loc)
            nc.gpsimd.collective_compute(
                "AllReduce", ADD,
                replica_groups=[list(range(NC))],
                ins=[ar_in.opt()], outs=[ar_out.opt()],
            )
            h1r = sb.tile([128, 4, B], BF16, tag="h1r")
            nc.scalar.dma_start(h1r, ar_out)
            h1T = []
            for ht in range(4):
                t = sb.tile([128, B], BF16, tag=f"h1T{ht}", name=f"h1T{ht}")
                nc.scalar.activation(t, h1r[:, ht, :], RELU,
                                     bias=b1sb[:, ht:ht + 1])
                h1T.append(t)

            # ---------- fc2 (redundant on every core, trivial) ----------
            hT = []
            for ht in range(4):
                p = psfc.tile([128, B], F32, tag="fc", name=f"ps2_{ht}")
                for kc in range(4):
                    nc.tensor.matmul(p, w2sb[:, kc * H + ht * 128:kc * H + (ht + 1) * 128],
                                     h1T[kc],
                                     start=(kc == 0), stop=(kc == 3))
                t = sb.tile([128, B], BF16, tag=f"hT{ht}", name=f"hT{ht}")
                nc.scalar.activation(t, p, RELU, bias=b2sb[:, ht:ht + 1])
                hT.append(t)

            # ---------- assoc heads + value_c (augmented matmul) ----------
            ar1_in = sb.tile([128, 2], F32, tag="ar1_in")
            ar2_in = sb.tile([128, 2], F32, tag="ar2_in")
            value_sb = []
            junkA = sb.tile([128, M], F32, tag="junkA")
            for bt in range(2):
                psA = psw.tile([128, AUG], F32, tag="wide", name=f"psA{bt}")
                nc.tensor.matmul(psA, ones1, augbsb, start=True, stop=False)
                for kc in range(4):
                    nc.tensor.matmul(psA, hT[kc][:, ts(bt, 128)],
                                     augsb[:, kc * AUG:(kc + 1) * AUG],
                                     start=False, stop=(kc == 3))
                psA_sb = sb.tile([128, AUG], F32, tag=f"psAsb{bt}", name=f"psAsb{bt}")
                nc.scalar.copy(psA_sb, psA)
                advs = psA_sb[:, 0:HPC * (M + 1)].rearrange("p (n u) -> p n u", u=M + 1)
                negm = sb.tile([128, HPC], F32, tag=f"negmA{bt}", name=f"negmA{bt}")
                for n in range(HPC):
                    nc.scalar.activation(junkA, advs[:, n, 0:M], COPY,
                                         scale=-1.0 / M,
                                         accum_out=negm[:, n:n + 1])
                tmp = sb.tile([128, HPC], F32, tag=f"tmpA{bt}", name=f"tmpA{bt}")
                nc.gpsimd.tensor_add(tmp, advs[:, :, M], negm)
                q = sb.tile([128, HPC * M], F32, tag=f"qA{bt}", name=f"qA{bt}")
                nc.gpsimd.tensor_tensor(
                    out=q.rearrange("p (n m) -> p n m", m=M),
                    in0=advs[:, :, 0:M],
                    in1=tmp.broadcast_to([128, HPC, M]),
                    op=ADD)
                nc.scalar.dma_start(out_assoc[ts(bt, 128), :], q)
                value_sb.append(psA_sb[:, AUG - 1:AUG])

            # ---------- cache head (R-slice) + early row-sums ----------
            adv_c_sb = []
            for bt in range(2):
                psC = psw.tile([128, RPC], F32, tag="wide", name=f"psC{bt}")
                nc.tensor.matmul(psC, ones1, bacsb, start=True, stop=False)
                for kc in range(4):
                    nc.tensor.matmul(psC, hT[kc][:, ts(bt, 128)],
                                     wacsb[:, ts(kc, RPC)],
                                     start=False, stop=(kc == 3))
                t = sb.tile([128, RPC], F32, tag=f"advc{bt}", name=f"advc{bt}")
                nc.scalar.activation(t, psC, COPY,
                                     accum_out=ar1_in[:, bt:bt + 1])
                adv_c_sb.append(t)

            # tiny AllGather #1 (cache path; hidden under the stream)
            ag1_din = dram.tile([128, 2], F32, tag="ag1_din")
            ag1_dout = dram.tile([NC * 128, 2], F32, tag="ag1_dout",
                                 addr_space="Shared")
            nc.scalar.dma_start(ag1_din, ar1_in)
            nc.gpsimd.collective_compute(
                "AllGather", mybir.AluOpType.bypass,
                replica_groups=[list(range(NC))],
                ins=[ag1_din.opt()], outs=[ag1_dout.opt()],
            )
            rall1 = sb.tile([128, NC * 2], F32, tag="rall1")
            nc.scalar.dma_start(rall1, ag1_dout.rearrange("(g p) c -> p g c", p=128))
            junk8 = sb.tile([128, NC], F32, tag="junk8")
            negm1 = sb.tile([128, 2], F32, tag="negm1")
            for bt in range(2):
                gv = bass.AP(rall1.tensor, rall1.offset + bt,
                             [rall1.ap[0], [2, NC]])
                nc.scalar.activation(junk8, gv, COPY, scale=-1.0 / R,
                                     accum_out=negm1[:, bt:bt + 1])
            for bt in range(2):
                vm = sb.tile([128, 1], F32, tag=f"vm{bt}", name=f"vm{bt}")
                nc.gpsimd.tensor_add(vm, value_sb[bt], negm1[:, bt:bt + 1])
                qc = sb.tile([128, RPC], F32, tag=f"qc{bt}", name=f"qc{bt}")
                nc.scalar.activation(qc, adv_c_sb[bt], IDENT, bias=vm, scale=1.0)
                nc.scalar.dma_start(out_cache[ts(bt, 128), :], qc)

            # ---------- S = hT.T @ acc (+ sum_n bru), interleaved by kc ------
            psS = []
            for bt in range(2):
                p = pss.tile([128, RPC], F32, tag="s", name=f"psS{bt}")
                nc.tensor.matmul(p, ones64, brusb, start=True, stop=False)
                psS.append(p)
            for kc in range(4):
                for bt in range(2):
                    nc.tensor.matmul(psS[bt], hT[kc][:, ts(bt, 128)], acc[kc],
                                     start=False, stop=(kc == 3))
            s_sb = []
            for bt in range(2):
                st = sb.tile([128, RPC], F32, tag=f"ssb{bt}", name=f"ssb{bt}")
                nc.scalar.activation(st, psS[bt], COPY,
                                     accum_out=ar2_in[:, bt:bt + 1])
                s_sb.append(st)

            # tiny AllGather #2 (rec path; the only exposed tail collective)
            ag2_din = dram.tile([128, 2], F32, tag="ag2_din")
            ag2_dout = dram.tile([NC * 128, 2], F32, tag="ag2_dout",
                                 addr_space="Shared")
            nc.scalar.dma_start(ag2_din, ar2_in)
            nc.gpsimd.collective_compute(
                "AllGather", mybir.AluOpType.bypass,
                replica_groups=[list(range(NC))],
                ins=[ag2_din.opt()], outs=[ag2_dout.opt()],
            )
            rall2 = sb.tile([128, NC * 2], F32, tag="rall2")
            nc.scalar.dma_start(rall2, ag2_dout.rearrange("(g p) c -> p g c", p=128))
            negm2 = sb.tile([128, 2], F32, tag="negm2")
            for bt in range(2):
                gv = bass.AP(rall2.tensor, rall2.offset + bt,
                             [rall2.ap[0], [2, NC]])
                nc.scalar.activation(junk8, gv, COPY, scale=-1.0 / R,
                                     accum_out=negm2[:, bt:bt + 1])
            for bt in range(2):
                qr = sb.tile([128, RPC], F32, tag=f"qr{bt}", name=f"qr{bt}")
                nc.scalar.activation(qr, s_sb[bt], IDENT,
                                     bias=negm2[:, bt:bt + 1], scale=1.0)
                nc.scalar.dma_start(out_rec[ts(bt, 128), :], qr)

    nc.compile()
    return nc


_CACHED = None


def _get_program():
    global _CACHED
    if _CACHED is None:
        _CACHED = build_program()
    return _CACHED


def make_in_maps(x, W1, b1, W2, b2, Wvc, bvc, Wac, bac, Wvu, bvu, Wau, bau, Wru, bru):
    f = np.float32
    x_bf = np.asarray(x, f).astype(BF)                    # [B, STATE]
    W1_bf = np.asarray(W1, f).astype(BF)                  # [STATE, H]
    W2_bf = np.asarray(W2, f).astype(BF)
    Wac_bf = np.asarray(Wac, f).astype(BF)
    Wru_bf = np.asarray(Wru, f).astype(BF)                # [64, 512, 4000]
    Wau_f = np.asarray(Wau, f)
    Wvu_f = np.asarray(Wvu, f)
    Wvc_f = np.asarray(Wvc, f).reshape(H)
    bau_f = np.asarray(bau, f)
    bvu_f = np.asarray(bvu, f)
    bvc_f = np.asarray(bvc, f).reshape(1)
    bru_bf = np.asarray(bru, f).astype(BF)
    bac_f = np.asarray(bac, f)
    b1_f = np.asarray(b1, f)
    b2_f = np.asarray(b2, f)

    # w2: [h1, h2] -> [p, kc*H + h2]
    w2p = np.ascontiguousarray(
        W2_bf.reshape(4, 128, H).transpose(1, 0, 2)).reshape(128, 4 * H)
    b1p = np.ascontiguousarray(b1_f.reshape(4, 128).T)
    b2p = np.ascontiguousarray(b2_f.reshape(4, 128).T)

    in_maps = []
    for c in range(NC):
        k0 = c * KPC_RAW
        r0 = c * RPC
        h0 = c * HPC
        # xT: [p, kc*B + b] = x[b, k0 + kc*128 + p]
        xs = np.zeros((KPC, B), BF)
        xs[:KPC_RAW] = x_bf[:, k0:k0 + KPC_RAW].T
        xt = np.ascontiguousarray(
            xs.reshape(KCH, 128, B).transpose(1, 0, 2)).reshape(128, KCH * B)
        # w1: [p, kc*H + h] = W1[k0 + kc*128 + p, h]
        w1s = np.zeros((KPC, H), BF)
        w1s[:KPC_RAW] = W1_bf[k0:k0 + KPC_RAW]
        w1p_ = np.ascontiguousarray(
            w1s.reshape(KCH, 128, H).transpose(1, 0, 2)).reshape(128, KCH * H)
        # wac: [p, kc*RPC + r] = Wac[kc*128 + p, r0 + r]
        wacp_ = np.ascontiguousarray(
            Wac_bf[:, r0:r0 + RPC].reshape(4, 128, RPC).transpose(1, 0, 2)
        ).reshape(128, 4 * RPC)
        # wru supertiles, head-block layout (contiguous blocks for the DVE
        # TT tree): [kc, g, p, blk*RPC + r] = Wru[g*GRP+blk, kc*128+p, r0+r]
        ws = Wru_bf[:, :, r0:r0 + RPC]                    # [64, 512, 500]
        a = ws.reshape(NGRP, GRP, 4, 128, RPC)            # [g, blk, kc, p, r]
        wru_p = np.ascontiguousarray(a.transpose(2, 0, 3, 1, 4)).reshape(
            4, NGRP, 128, GRP * RPC)
        # aug weights: [k, n*(M+1)+m | n*(M+1)+M | 88]
        aug_full = np.empty((H, AUG), f)
        aug_full[:, 0:HPC * (M + 1)] = np.concatenate(
            [Wau_f[h0:h0 + HPC].transpose(1, 0, 2),            # [H, 8, 10]
             Wvu_f[h0:h0 + HPC].T[:, :, None]], axis=2         # [H, 8, 1]
        ).reshape(H, HPC * (M + 1))
        aug_full[:, AUG - 1] = Wvc_f
        augp_ = np.ascontiguousarray(
            aug_full.astype(BF).reshape(4, 128, AUG).transpose(1, 0, 2)
        ).reshape(128, 4 * AUG)
        augb_ = np.empty((1, AUG), f)
        augb_[0, 0:HPC * (M + 1)] = np.concatenate(
            [bau_f[h0:h0 + HPC], bvu_f[h0:h0 + HPC, None]], axis=1
        ).reshape(HPC * (M + 1))
        augb_[0, AUG - 1] = bvc_f[0]
        m = {
            "xt": xt,
            "w1": w1p_,
            "b1p": b1p,
            "w2p": w2p,
            "b2p": b2p,
            "wacp": wacp_,
            "bacp": np.ascontiguousarray(bac_f[None, r0:r0 + RPC]).astype(BF),
            "wrup": wru_p,
            "brup": np.ascontiguousarray(bru_bf[:, r0:r0 + RPC]),
            "augp": augp_,
            "augb": augb_.astype(BF),
        }
        in_maps.append(m)
    return in_maps


def assemble(results):
    q = np.empty((B, 2 * R + NH * M), np.float32)
    for c in range(NC):
        r0 = c * RPC
        a0 = c * HPC * M
        q[:, r0:r0 + RPC] = results[c]["out_cache"]
        q[:, R + r0:R + r0 + RPC] = results[c]["out_rec"]
        q[:, 2 * R + a0:2 * R + a0 + HPC * M] = results[c]["out_assoc"]
    return q


def run(in_maps, **kw):
    nc = _get_program()
    return bass_utils.run_bass_kernel_spmd(nc, in_maps, core_ids=list(range(NC)), **kw)


def kernel(**inputs):
    in_maps = make_in_maps(**{k: np.asarray(v) for k, v in inputs.items()})
    res = run(in_maps)
    return assemble(res.results)
